# revision 1
# baseline (speedup 1.0000x reference)
"""Trainium2 Bass kernel for the DistancePositionOperator.

Reference computation (B=2, L=1024, D=128):
    delta[b,i,j,:] = X[b,i,:] - X[b,j,:]
    alpha[i,j]     = 1 / (1 + |i-j|)            (zero on the diagonal)
    d[b,i,j]       = sum_d |delta|              (pairwise L1 distance)
    C[b,i,j]       = alpha[i,j] / (1 + d[b,i,j])
    O[b,i,:]       = sum_j C[b,i,j] * delta[b,i,j,:]
                   = rowsum(C)[b,i] * X[b,i,:] - (C @ X)[b,i,:]

d and C are symmetric in (i,j), so only one of each 128x128 block pair
is computed: with L split into 8 strip-blocks that is 36 blocks per
batch, 72 total -> 9 per core.  Core q (batch q//4, q%4 -> rotation)
computes blocks (I, (I+K) mod 8) for K in 0..3 at I in {q, q+4} plus the
(q, q+4) anti-diagonal block.  Host-side each core's inputs are rotated
by 128*q tokens so every core runs the IDENTICAL program: strip 0
against key span [0,640) and strip 4 against [512,1024), both
contiguous.  The host un-rotates and sums the per-core partial outputs.

Per query row i the elementwise engines (ACT: Abs activation with
per-partition bias; DVE: custom |x - c| uop chain) emit
Abs_i[d, jspan] in bf16.  The PE reduces over d by using Abs_i as
matmul *weights* against a ones vector, landing dT[j, i] directly in
PSUM in the transposed layout needed downstream.  C^T = alpha^T/(1+dT)
then drives output matmuls (C^T as weights, [X | 1] as moving) which
produce C@X and rowsum(C) in one pass; the mirror contribution uses the
PE-transposed C block.
"""

import numpy as np
import ml_dtypes

B, L, D = 2, 1024, 128
NBLK = L // 128                      # 8 strip blocks per batch
N_CORES = 8
# per-strip i -> engine split (ACT, DVE, Pool): balances engine busy time;
# Pool uses a 2-instruction relu/min path with +-ones PE accumulation.
STRIP_SPLIT = {0: (58, 51, 19), 4: (60, 49, 19)}


def _engine_pattern(n_act, n_dve, n_pool):
    counts = [n_act, n_dve, n_pool]
    acc = [0.0, 0.0, 0.0]
    pat = []
    for _ in range(sum(counts)):
        for e in range(3):
            acc[e] += counts[e]
        e = max(range(3), key=lambda k: acc[k])
        acc[e] -= sum(counts)
        pat.append("AVP"[e])
    return pat

# program-relative schedule (identical on every core):
#   (query strip, [key blocks])
SCHED = [(0, [0, 1, 2, 3, 4]), (4, [4, 5, 6, 7])]
NBLOCKS = 9

_COMPILED = None


def _register_abs_diff():
    import concourse.dve_ops as dve_ops
    from concourse.dve_spec import Spec, Src0, C0, maxx, lower as dve_lower, _has_src1
    from concourse.dve_uop import DveOpSpec

    name = "ABS_DIFF_ANT_X"
    for op in dve_ops.OPS:
        if op.name == name:
            return op
    spec = Spec(
        body=maxx(Src0 - C0, C0 - Src0),
        reference=lambda in0, in1, s0, s1, imm2: np.abs(in0.astype(np.float32) - s0),
    )
    row = max(dve_ops._SUB_OPCODE_FOR_NAME.values()) + 1
    shas = {}
    for ver in ("v3", "v4"):
        s = DveOpSpec(name=name, opcode=row, uops=dve_lower(spec, ver=ver),
                      rd1_en=_has_src1(spec))
        shas[ver] = s.sha(ver)
    op = dve_ops.DveOp(name, spec, subdim=False, uops_sha=shas)
    dve_ops.OPS.append(op)
    dve_ops._SUB_OPCODE_FOR_NAME[name] = row
    dve_ops.CUSTOM_DVE_SPECS[name] = spec
    return op


def _build(iters=1):
    """Build + compile the (core-uniform) Bass program."""
    import concourse.bacc as bacc
    import concourse.tile as tile
    import concourse.mybir as mybir
    from concourse.masks import make_identity

    ABS_DIFF = _register_abs_diff()
    F32, BF16 = mybir.dt.float32, mybir.dt.bfloat16
    AF = mybir.ActivationFunctionType
    ALU = mybir.AluOpType

    nc = bacc.Bacc("TRN2", target_bir_lowering=False, debug=False,
                   num_devices=N_CORES)
    xt16_ap = nc.dram_tensor("xt16", [D, L], BF16, kind="ExternalInput").ap()
    xt32_ap = nc.dram_tensor("xt32b", [D, 256], F32, kind="ExternalInput").ap()
    xaug_ap = nc.dram_tensor("xaug", [NBLK, 128, D + 1], F32,
                             kind="ExternalInput").ap()
    alpha_ap = nc.dram_tensor("alphat", [NBLOCKS, 128, 128], F32,
                              kind="ExternalInput").ap()
    pout_ap = nc.dram_tensor("pout", [NBLK, 128, D], F32,
                             kind="ExternalOutput").ap()

    with tile.TileContext(nc) as tc:
        with tc.tile_pool(name="consts", bufs=1) as consts, \
             tc.tile_pool(name="abs", bufs=14) as abs_pool, \
             tc.tile_pool(name="work", bufs=3) as work, \
             tc.tile_pool(name="dtps", bufs=5, space="PSUM") as dtps, \
             tc.tile_pool(name="ops", bufs=2, space="PSUM") as ops_ps, \
             tc.tile_pool(name="tps", bufs=1, space="PSUM") as tps:

            xt16 = consts.tile([D, L], BF16, tag="xt16")
            xt32 = consts.tile([D, 256], F32, tag="xt32")
            nc.sync.dma_start(xt32[:, 0:128], xt32_ap[:, 0:128])
            nc.sync.dma_start(xt16[:, 0:640], xt16_ap[:, 0:640])
            nc.sync.dma_start(xt32[:, 128:256], xt32_ap[:, 128:256])
            nc.sync.dma_start(xt16[:, 640:L], xt16_ap[:, 640:L])
            xaug = consts.tile([128, NBLK * (D + 1)], F32, tag="xaug")
            for s in range(NBLK):
                nc.sync.dma_start(xaug[:, s * (D + 1):(s + 1) * (D + 1)],
                                  xaug_ap[s])
            alpha = consts.tile([128, NBLOCKS * 128], F32, tag="alpha")
            for k in range(NBLOCKS):
                nc.sync.dma_start(alpha[:, k * 128:(k + 1) * 128], alpha_ap[k])
            ones16 = consts.tile([D, 1], BF16, tag="ones")
            nc.vector.memset(ones16[:], 1.0)
            nones16 = consts.tile([D, 1], BF16, tag="nones")
            nc.vector.memset(nones16[:], -1.0)
            ident = consts.tile([128, 128], F32, tag="ident")
            make_identity(nc, ident[:])

            import contextlib
            loop_cm = (tc.For_i(0, iters, 1) if iters > 1
                       else contextlib.nullcontext())
            with loop_cm:
                _kernel_body(nc, tc, mybir, ABS_DIFF, xt16, xt32, xaug, alpha,
                             ones16, nones16, ident, consts, abs_pool, work,
                             dtps, ops_ps, tps, pout_ap)

    nc.compile()
    return nc


def _kernel_body(nc, tc, mybir, ABS_DIFF, xt16, xt32, xaug, alpha, ones16,
                 nones16, ident, consts, abs_pool, work, dtps, ops_ps, tps,
                 pout_ap):
    F32, BF16 = mybir.dt.float32, mybir.dt.bfloat16
    AF = mybir.ActivationFunctionType
    ALU = mybir.AluOpType
    if True:
        if True:
            oacc = []
            for s in range(NBLK):
                t = consts.tile([128, D + 1], F32, tag=f"oacc{s}")
                nc.gpsimd.memset(t[:], 0.0)
                oacc.append(t)

            def xaug_blk(s):
                return xaug[:, s * (D + 1):(s + 1) * (D + 1)]

            blk = 0
            for R, jblocks in SCHED:
                nb = len(jblocks)
                jlo = jblocks[0] * 128
                w = nb * 128
                dts = [dtps.tile([128, 128], F32, tag="dt", name=f"dt{R}_{k}")
                       for k in range(nb)]
                pat = _engine_pattern(*STRIP_SPLIT[R])
                boff = 0 if R == 0 else 128
                for i in range(128):
                    gi = boff + i
                    eng = pat[i]
                    if eng == "P":
                        pp = abs_pool.tile([D, w], BF16, tag="ab", name=f"pp{R}_{i}")
                        nc.gpsimd.tensor_scalar(
                            pp[:], xt16[:, jlo:jlo + w], xt32[:, gi:gi + 1],
                            0.0, ALU.subtract, ALU.max)
                        mm = abs_pool.tile([D, w], BF16, tag="ab", name=f"mm{R}_{i}")
                        nc.gpsimd.tensor_scalar(
                            mm[:], xt16[:, jlo:jlo + w], xt32[:, gi:gi + 1],
                            0.0, ALU.subtract, ALU.min)
                        for k in range(nb):
                            nc.tensor.matmul(
                                dts[k][:, i:i + 1],
                                lhsT=pp[:, k * 128:(k + 1) * 128],
                                rhs=ones16[:], start=True, stop=False)
                            nc.tensor.matmul(
                                dts[k][:, i:i + 1],
                                lhsT=mm[:, k * 128:(k + 1) * 128],
                                rhs=nones16[:], start=False, stop=True)
                        continue
                    ab = abs_pool.tile([D, w], BF16, tag="ab", name=f"ab{R}_{i}")
                    if eng == "A":
                        nc.scalar.activation(
                            ab[:], xt16[:, jlo:jlo + w], AF.Abs,
                            bias=xt32[:, gi:gi + 1], scale=-1.0)
                    else:
                        nc.vector._custom_dve(
                            ABS_DIFF, out=ab[:], in0=xt16[:, jlo:jlo + w],
                            s0=xt32[:, gi:gi + 1])
                    for k in range(nb):
                        nc.tensor.matmul(
                            dts[k][:, i:i + 1],
                            lhsT=ab[:, k * 128:(k + 1) * 128],
                            rhs=ones16[:], start=True, stop=True)
                for k, J in enumerate(jblocks):
                    # u = 1 + dT   (PSUM -> SBUF, frees the dt bank)
                    u = work.tile([128, 128], F32, tag="u")
                    nc.vector.tensor_scalar_add(u[:], dts[k][:], 1.0)
                    r = work.tile([128, 128], F32, tag="r")
                    nc.vector.reciprocal_approx_fast(r[:], u[:])
                    ct = work.tile([128, 128], F32, tag="ct")
                    nc.gpsimd.tensor_tensor(
                        ct[:], r[:], alpha[:, blk * 128:(blk + 1) * 128],
                        ALU.mult)
                    # O_R partial: [C@X | rowsum] over keys in block J
                    po = ops_ps.tile([128, D + 1], F32, tag="po")
                    nc.tensor.matmul(po[:], lhsT=ct[:], rhs=xaug_blk(J),
                                     start=True, stop=True)
                    nc.vector.tensor_add(oacc[R][:], oacc[R][:], po[:])
                    if J != R:
                        pt = tps.tile([128, 128], F32, tag="pt")
                        nc.tensor.transpose(pt[:], ct[:], ident[:])
                        ctT = work.tile([128, 128], F32, tag="ctT")
                        nc.scalar.copy(ctT[:], pt[:])
                        po2 = ops_ps.tile([128, D + 1], F32, tag="po")
                        nc.tensor.matmul(po2[:], lhsT=ctT[:],
                                         rhs=xaug_blk(R),
                                         start=True, stop=True)
                        nc.vector.tensor_add(oacc[J][:], oacc[J][:], po2[:])
                    blk += 1

            # O_s = rowsum * X_s - (C@X)_s  =  X_s*oacc[:,D] - oacc[:,:D]
            for s in range(NBLK):
                o = work.tile([128, D], F32, tag="fin")
                nc.vector.scalar_tensor_tensor(
                    o[:], xaug_blk(s)[:, 0:D], oacc[s][:, D:D + 1],
                    oacc[s][:, 0:D], ALU.mult, ALU.subtract)
                nc.sync.dma_start(pout_ap[s], o[:])


_ALPHA_CACHE = {}


def _core_alpha(q):
    if q in _ALPHA_CACHE:
        return _ALPHA_CACHE[q]
    idx = np.arange(L, dtype=np.float64)
    rot = 128 * q
    real = (idx + rot) % L
    al = np.empty((NBLOCKS, 128, 128), dtype=np.float32)
    k = 0
    for R, jblocks in SCHED:
        ti = real[R * 128:(R + 1) * 128]
        for J in jblocks:
            tj = real[J * 128:(J + 1) * 128]
            dist = np.abs(tj[:, None] - ti[None, :])
            a = 1.0 / (1.0 + dist)
            a[dist == 0] = 0.0
            al[k] = a.astype(np.float32)
            k += 1
    _ALPHA_CACHE[q] = al
    return al


def _prep_host(X):
    """Per-core rotated input dicts. X: [B, L, D] float32."""
    in_maps = []
    for c in range(N_CORES):
        b, q = c // 4, c % 4
        rot = 128 * q
        Xr = np.roll(X[b], -rot, axis=0)          # program token t = real t+rot
        xtT = np.ascontiguousarray(Xr.T)          # [D, L]
        xt16 = xtT.astype(ml_dtypes.bfloat16)
        xt32b = np.ascontiguousarray(
            np.concatenate([xtT[:, 0:128], xtT[:, 512:640]], axis=1))
        xaug = np.concatenate(
            [Xr, np.ones((L, 1), dtype=np.float32)], axis=1)
        xaug = np.ascontiguousarray(xaug.reshape(NBLK, 128, D + 1))
        in_maps.append({"xt16": xt16, "xt32b": xt32b, "xaug": xaug,
                        "alphat": _core_alpha(q)})
    return in_maps


def _get_compiled():
    global _COMPILED
    if _COMPILED is None:
        _COMPILED = _build()
    return _COMPILED


def kernel(X, _trace=False, _trace_kwargs=None):
    """X: np.ndarray [2, 1024, 128] float32 -> O [2, 1024, 128] float32."""
    from concourse.bass_utils import run_bass_kernel_spmd

    X = np.asarray(X, dtype=np.float32)
    assert X.shape == (B, L, D)
    nc = _get_compiled()
    in_maps = _prep_host(X)
    res = run_bass_kernel_spmd(nc, in_maps, list(range(N_CORES)),
                               trace=_trace, **(_trace_kwargs or {}))
    O = np.zeros((B, L, D), dtype=np.float32)
    for c in range(N_CORES):
        b, q = c // 4, c % 4
        part = res.results[c]["pout"].reshape(L, D)
        O[b] += np.roll(part, 128 * q, axis=0)    # un-rotate
    if _trace:
        return O, res
    return O


if __name__ == "__main__":
    rng = np.random.default_rng(0)
    X = rng.standard_normal((B, L, D), dtype=np.float32)
    O = kernel(X)
    print("ok", O.shape, float(np.abs(O).max()))



# revision 10
# speedup vs baseline: 6.0081x; 6.0081x over previous
"""Trainium2 Bass kernel for the DistancePositionOperator.

Reference computation (B=2, L=1024, D=128):
    delta[b,i,j,:] = X[b,i,:] - X[b,j,:]
    alpha[i,j]     = 1 / (1 + |i-j|)            (zero on the diagonal)
    d[b,i,j]       = sum_d |delta|              (pairwise L1 distance)
    C[b,i,j]       = alpha[i,j] / (1 + d[b,i,j])
    O[b,i,:]       = sum_j C[b,i,j] * delta[b,i,j,:]
                   = rowsum(C)[b,i] * X[b,i,:] - (C @ X)[b,i,:]

d and C are symmetric in (i,j), so only one of each 128x128 block pair
is computed: with L split into 8 strip-blocks that is 36 blocks per
batch, 72 total -> 9 per core.  Core q (batch q//4, q%4 -> rotation)
computes blocks (I, (I+K) mod 8) for K in 0..3 at I in {q, q+4} plus the
(q, q+4) anti-diagonal block.  Host-side each core's inputs are rotated
by 128*q tokens so every core runs the IDENTICAL program: strip 0
against key span [0,640) and strip 4 against [512,1024), both
contiguous.  The host un-rotates and sums the per-core partial outputs.

Per query row i the elementwise engines compute Abs_i[d, jspan] in bf16:
ACT via Abs activation with per-partition bias, DVE via the standard
tensor_scalar (x - c) abs_max 0 (supports the fast DVE perf modes).
The PE reduces over d by using Abs_i as matmul *weights* against a ones
vector, landing dT[j, i] directly in PSUM in the transposed layout
needed downstream.  C^T = alpha^T/(1+dT) then drives output matmuls
(C^T as weights, [X | 1] as moving) which produce C@X and rowsum(C) in
one pass; the mirror contribution uses the PE-transposed C block.
"""

import numpy as np
import ml_dtypes

B, L, D = 2, 1024, 128
NBLK = L // 128                      # 8 strip blocks per batch
N_CORES = 8
# per-strip i -> engine split (ACT, DVE): ACT Abs ~718ns/row (640w), DVE
# relu/min native tensor_scalar pair ~592ns/row (2x perf mode).
STRIP_SPLIT = {0: (58, 70), 4: (59, 69)}

# program-relative schedule (identical on every core):
#   (query strip, [key blocks])
SCHED = [(0, [0, 1, 2, 3, 4]), (4, [4, 5, 6, 7])]
NBLOCKS = 9

_COMPILED = None


def _engine_pattern(n_act, n_dve):
    counts = [n_act, n_dve]
    acc = [0.0, 0.0]
    pat = []
    for _ in range(sum(counts)):
        for e in range(2):
            acc[e] += counts[e]
        e = max(range(2), key=lambda k: acc[k])
        acc[e] -= sum(counts)
        pat.append("AV"[e])
    return pat


def _build(iters=1):
    """Build + compile the (core-uniform) Bass program."""
    import concourse.bacc as bacc
    import concourse.tile as tile
    import concourse.mybir as mybir
    from concourse.masks import make_identity

    F32, BF16 = mybir.dt.float32, mybir.dt.bfloat16
    AF = mybir.ActivationFunctionType
    ALU = mybir.AluOpType

    nc = bacc.Bacc("TRN2", target_bir_lowering=False, debug=False,
                   num_devices=N_CORES)
    xt16_ap = nc.dram_tensor("xt16", [D, L], BF16, kind="ExternalInput").ap()
    xt32_ap = nc.dram_tensor("xt32b", [D, 256], F32, kind="ExternalInput").ap()
    xaug_ap = nc.dram_tensor("xaug", [NBLK, 128, D + 1], F32,
                             kind="ExternalInput").ap()
    alpha_ap = nc.dram_tensor("alphat", [NBLOCKS, 128, 128], F32,
                              kind="ExternalInput").ap()
    pout_ap = nc.dram_tensor("pout", [NBLK, 128, D], F32,
                             kind="ExternalOutput").ap()

    with tile.TileContext(nc) as tc:
        with tc.tile_pool(name="consts", bufs=1) as consts, \
             tc.tile_pool(name="abs", bufs=16) as abs_pool, \
             tc.tile_pool(name="work", bufs=3) as work, \
             tc.tile_pool(name="dtps", bufs=5, space="PSUM") as dtps, \
             tc.tile_pool(name="ops", bufs=2, space="PSUM") as ops_ps, \
             tc.tile_pool(name="tps", bufs=1, space="PSUM") as tps:

            xt16 = consts.tile([D, L], BF16, tag="xt16")
            xt32 = consts.tile([D, 256], F32, tag="xt32")
            nc.sync.dma_start(xt32[:, 0:128], xt32_ap[:, 0:128])
            nc.sync.dma_start(xt16[:, 0:640], xt16_ap[:, 0:640])
            nc.sync.dma_start(xt32[:, 128:256], xt32_ap[:, 128:256])
            nc.sync.dma_start(xt16[:, 640:L], xt16_ap[:, 640:L])
            xaug = consts.tile([128, NBLK * (D + 1)], F32, tag="xaug")
            for s in range(NBLK):
                nc.sync.dma_start(xaug[:, s * (D + 1):(s + 1) * (D + 1)],
                                  xaug_ap[s])
            alpha = consts.tile([128, NBLOCKS * 128], F32, tag="alpha")
            for k in range(NBLOCKS):
                nc.sync.dma_start(alpha[:, k * 128:(k + 1) * 128], alpha_ap[k])
            ones16 = consts.tile([D, 1], BF16, tag="ones")
            nc.vector.memset(ones16[:], 1.0)
            nones16 = consts.tile([D, 1], BF16, tag="nones")
            nc.vector.memset(nones16[:], -1.0)
            ident = consts.tile([128, 128], F32, tag="ident")
            make_identity(nc, ident[:])

            import contextlib
            loop_cm = (tc.For_i(0, iters, 1) if iters > 1
                       else contextlib.nullcontext())
            with loop_cm:
                _kernel_body(nc, tc, mybir, xt16, xt32, xaug, alpha,
                             ones16, nones16, ident, consts, abs_pool, work,
                             dtps, ops_ps, tps, pout_ap)

    nc.compile()
    return nc


def _kernel_body(nc, tc, mybir, xt16, xt32, xaug, alpha, ones16, nones16,
                 ident, consts, abs_pool, work, dtps, ops_ps, tps,
                 pout_ap):
    F32, BF16 = mybir.dt.float32, mybir.dt.bfloat16
    AF = mybir.ActivationFunctionType
    ALU = mybir.AluOpType

    oacc = []
    for s in range(NBLK):
        t = consts.tile([128, D + 1], F32, tag=f"oacc{s}")
        nc.gpsimd.memset(t[:], 0.0)
        oacc.append(t)

    def xaug_blk(s):
        return xaug[:, s * (D + 1):(s + 1) * (D + 1)]

    fin_count = [0]

    def finalize(s):
        # O_s = rowsum * X_s - (C@X)_s  =  X_s*oacc[:,D] - oacc[:,:D]
        o = work.tile([128, D], F32, tag="fin")
        fin_count[0] += 1
        nc.vector.scalar_tensor_tensor(
            o[:], xaug_blk(s)[:, 0:D], oacc[s][:, D:D + 1],
            oacc[s][:, 0:D], ALU.mult, ALU.subtract)
        nc.sync.dma_start(pout_ap[s], o[:])

    blk = 0
    for R, jblocks in SCHED:
        nb = len(jblocks)
        jlo = jblocks[0] * 128
        w = nb * 128
        dts = [dtps.tile([128, 128], F32, tag="dt", name=f"dt{R}_{k}")
               for k in range(nb)]
        pat = _engine_pattern(*STRIP_SPLIT[R])
        boff = 0 if R == 0 else 128
        for i in range(128):
            gi = boff + i
            if pat[i] == "A":
                ab = abs_pool.tile([D, w], BF16, tag="ab", name=f"ab{R}_{i}")
                nc.scalar.activation(
                    ab[:], xt16[:, jlo:jlo + w], AF.Abs,
                    bias=xt32[:, gi:gi + 1], scale=-1.0)
                for k in range(nb):
                    nc.tensor.matmul(
                        dts[k][:, i:i + 1],
                        lhsT=ab[:, k * 128:(k + 1) * 128],
                        rhs=ones16[:], start=True, stop=True)
            else:
                pp = abs_pool.tile([D, w], BF16, tag="ab", name=f"pp{R}_{i}")
                nc.vector.tensor_scalar(
                    pp[:], xt16[:, jlo:jlo + w], xt32[:, gi:gi + 1],
                    0.0, ALU.subtract, ALU.max)
                mm = abs_pool.tile([D, w], BF16, tag="ab", name=f"mm{R}_{i}")
                nc.vector.tensor_scalar(
                    mm[:], xt16[:, jlo:jlo + w], xt32[:, gi:gi + 1],
                    0.0, ALU.subtract, ALU.min)
                for k in range(nb):
                    nc.tensor.matmul(
                        dts[k][:, i:i + 1],
                        lhsT=pp[:, k * 128:(k + 1) * 128],
                        rhs=ones16[:], start=True, stop=False)
                    nc.tensor.matmul(
                        dts[k][:, i:i + 1],
                        lhsT=mm[:, k * 128:(k + 1) * 128],
                        rhs=nones16[:], start=False, stop=True)
        for k, J in enumerate(jblocks):
            # u = 1 + dT   (PSUM -> SBUF, frees the dt bank); ACT engine
            u = work.tile([128, 128], F32, tag="u")
            nc.scalar.add(u[:], dts[k][:], 1.0)
            r = work.tile([128, 128], F32, tag="r")
            nc.vector.reciprocal_approx_fast(r[:], u[:])
            ct = work.tile([128, 128], F32, tag="ct")
            nc.gpsimd.tensor_tensor(
                ct[:], r[:], alpha[:, blk * 128:(blk + 1) * 128],
                ALU.mult)
            # O_R partial: [C@X | rowsum] over keys in block J
            po = ops_ps.tile([128, D + 1], F32, tag="po")
            nc.tensor.matmul(po[:], lhsT=ct[:], rhs=xaug_blk(J),
                             start=True, stop=True)
            nc.vector.tensor_tensor(oacc[R][:], oacc[R][:], po[:], ALU.add)
            if J != R:
                pt = tps.tile([128, 128], F32, tag="pt")
                nc.tensor.transpose(pt[:], ct[:], ident[:])
                ctT = work.tile([128, 128], F32, tag="ctT")
                nc.scalar.copy(ctT[:], pt[:])
                po2 = ops_ps.tile([128, D + 1], F32, tag="po")
                nc.tensor.matmul(po2[:], lhsT=ctT[:],
                                 rhs=xaug_blk(R),
                                 start=True, stop=True)
                nc.vector.tensor_tensor(oacc[J][:], oacc[J][:], po2[:],
                                        ALU.add)
            blk += 1
        if R == 0:
            # strips 0..3 are complete after the strip-0 pass
            for s in range(4):
                finalize(s)
        else:
            for s in range(4, NBLK):
                finalize(s)


_ALPHA_CACHE = {}


def _core_alpha(q):
    if q in _ALPHA_CACHE:
        return _ALPHA_CACHE[q]
    idx = np.arange(L, dtype=np.float64)
    rot = 128 * q
    real = (idx + rot) % L
    al = np.empty((NBLOCKS, 128, 128), dtype=np.float32)
    k = 0
    for R, jblocks in SCHED:
        ti = real[R * 128:(R + 1) * 128]
        for J in jblocks:
            tj = real[J * 128:(J + 1) * 128]
            dist = np.abs(tj[:, None] - ti[None, :])
            a = 1.0 / (1.0 + dist)
            a[dist == 0] = 0.0
            al[k] = a.astype(np.float32)
            k += 1
    _ALPHA_CACHE[q] = al
    return al


def _prep_host(X):
    """Per-core rotated input dicts. X: [B, L, D] float32."""
    in_maps = []
    for c in range(N_CORES):
        b, q = c // 4, c % 4
        rot = 128 * q
        Xr = np.roll(X[b], -rot, axis=0)          # program token t = real t+rot
        xtT = np.ascontiguousarray(Xr.T)          # [D, L]
        xt16 = xtT.astype(ml_dtypes.bfloat16)
        xt32b = np.ascontiguousarray(
            np.concatenate([xtT[:, 0:128], xtT[:, 512:640]], axis=1))
        xaug = np.concatenate(
            [Xr, np.ones((L, 1), dtype=np.float32)], axis=1)
        xaug = np.ascontiguousarray(xaug.reshape(NBLK, 128, D + 1))
        in_maps.append({"xt16": xt16, "xt32b": xt32b, "xaug": xaug,
                        "alphat": _core_alpha(q)})
    return in_maps


def _get_compiled():
    global _COMPILED
    if _COMPILED is None:
        _COMPILED = _build()
    return _COMPILED


def kernel(X, _trace=False, _trace_kwargs=None):
    """X: np.ndarray [2, 1024, 128] float32 -> O [2, 1024, 128] float32."""
    from concourse.bass_utils import run_bass_kernel_spmd

    X = np.asarray(X, dtype=np.float32)
    assert X.shape == (B, L, D)
    nc = _get_compiled()
    in_maps = _prep_host(X)
    res = run_bass_kernel_spmd(nc, in_maps, list(range(N_CORES)),
                               trace=_trace, **(_trace_kwargs or {}))
    O = np.zeros((B, L, D), dtype=np.float32)
    for c in range(N_CORES):
        b, q = c // 4, c % 4
        part = res.results[c]["pout"].reshape(L, D)
        O[b] += np.roll(part, 128 * q, axis=0)    # un-rotate
    if _trace:
        return O, res
    return O


if __name__ == "__main__":
    rng = np.random.default_rng(0)
    X = rng.standard_normal((B, L, D), dtype=np.float32)
    O = kernel(X)
    print("ok", O.shape, float(np.abs(O).max()))


# revision 15
# speedup vs baseline: 6.1956x; 1.0312x over previous
"""Trainium2 Bass kernel for the DistancePositionOperator.

Reference computation (B=2, L=1024, D=128):
    delta[b,i,j,:] = X[b,i,:] - X[b,j,:]
    alpha[i,j]     = 1 / (1 + |i-j|)            (zero on the diagonal)
    d[b,i,j]       = sum_d |delta|              (pairwise L1 distance)
    C[b,i,j]       = alpha[i,j] / (1 + d[b,i,j])
    O[b,i,:]       = sum_j C[b,i,j] * delta[b,i,j,:]
                   = rowsum(C)[b,i] * X[b,i,:] - (C @ X)[b,i,:]

d and C are symmetric in (i,j), so only one of each 128x128 block pair
is computed: with L split into 8 strip-blocks that is 36 blocks per
batch, 72 total -> 9 per core.  Core q (batch q//4, q%4 -> rotation)
computes blocks (I, (I+K) mod 8) for K in 0..3 at I in {q, q+4} plus the
(q, q+4) anti-diagonal block.  Host-side each core's inputs are rotated
by 128*q tokens so every core runs the IDENTICAL program: strip 0
against key span [0,640) and strip 4 against [512,1024), both
contiguous.  The host un-rotates and sums the per-core partial outputs.

Per query row i the elementwise engines compute Abs_i[d, jspan] in bf16:
ACT via Abs activation with per-partition bias, DVE via the standard
tensor_scalar (x - c) abs_max 0 (supports the fast DVE perf modes).
The PE reduces over d by using Abs_i as matmul *weights* against a ones
vector, landing dT[j, i] directly in PSUM in the transposed layout
needed downstream.  C^T = alpha^T/(1+dT) then drives output matmuls
(C^T as weights, [X | 1] as moving) which produce C@X and rowsum(C) in
one pass; the mirror contribution uses the PE-transposed C block.
"""

import numpy as np
import ml_dtypes

B, L, D = 2, 1024, 128
NBLK = L // 128                      # 8 strip blocks per batch
N_CORES = 8
# per-strip i -> engine split (ACT, DVE): ACT Abs ~718ns/row (640w), DVE
# relu/min native tensor_scalar pair ~592ns/row (2x perf mode).
STRIP_SPLIT = {0: (58, 70), 4: (59, 69)}

# program-relative schedule (identical on every core):
#   (query strip, [key blocks])
SCHED = [(0, [0, 1, 2, 3, 4]), (4, [4, 5, 6, 7])]
NBLOCKS = 9

_COMPILED = None


def _engine_pattern(n_act, n_dve):
    counts = [n_act, n_dve]
    acc = [0.0, 0.0]
    pat = []
    for _ in range(sum(counts)):
        for e in range(2):
            acc[e] += counts[e]
        e = max(range(2), key=lambda k: acc[k])
        acc[e] -= sum(counts)
        pat.append("AV"[e])
    return pat


def _build(iters=1):
    """Build + compile the (core-uniform) Bass program."""
    import concourse.bacc as bacc
    import concourse.tile as tile
    import concourse.mybir as mybir
    from concourse.masks import make_identity

    F32, BF16 = mybir.dt.float32, mybir.dt.bfloat16
    AF = mybir.ActivationFunctionType
    ALU = mybir.AluOpType

    nc = bacc.Bacc("TRN2", target_bir_lowering=False, debug=False,
                   num_devices=N_CORES)
    xt16_ap = nc.dram_tensor("xt16", [D, L], BF16, kind="ExternalInput").ap()
    xt32_ap = nc.dram_tensor("xt32b", [D, 256], F32, kind="ExternalInput").ap()
    xaug_ap = nc.dram_tensor("xaug", [NBLK, 128, D + 1], F32,
                             kind="ExternalInput").ap()
    alpha_ap = nc.dram_tensor("alphat", [NBLOCKS, 128, 128], F32,
                              kind="ExternalInput").ap()
    pout_ap = nc.dram_tensor("pout", [NBLK, 128, D], F32,
                             kind="ExternalOutput").ap()

    with tile.TileContext(nc) as tc:
        with tc.tile_pool(name="consts", bufs=1) as consts, \
             tc.tile_pool(name="abs", bufs=20) as abs_pool, \
             tc.tile_pool(name="work", bufs=3) as work, \
             tc.tile_pool(name="dtaps", bufs=2, space="PSUM") as dtaps, \
             tc.tile_pool(name="dtbps", bufs=1, space="PSUM") as dtbps, \
             tc.tile_pool(name="pops", bufs=2, space="PSUM") as pops, \
             tc.tile_pool(name="po2ps", bufs=2, space="PSUM") as po2ps, \
             tc.tile_pool(name="tps", bufs=1, space="PSUM") as tps:

            xt16 = consts.tile([D, L], BF16, tag="xt16")
            xt32 = consts.tile([D, 256], F32, tag="xt32")
            # parallel startup: spread the critical key-span DMAs over
            # several engine queues (SP / Pool / PE run distinct DMA rings)
            nc.sync.dma_start(xt32[:, 0:128], xt32_ap[:, 0:128])
            nc.sync.dma_start(xt16[:, 0:320], xt16_ap[:, 0:320])
            nc.gpsimd.dma_start(xt16[:, 320:640], xt16_ap[:, 320:640])
            nc.scalar.dma_start(xt16[:, 640:L], xt16_ap[:, 640:L])
            nc.scalar.dma_start(xt32[:, 128:256], xt32_ap[:, 128:256])
            xaug = consts.tile([128, NBLK * (D + 1)], F32, tag="xaug")
            for s in range(NBLK):
                eng = nc.gpsimd if s % 2 == 0 else nc.sync
                eng.dma_start(xaug[:, s * (D + 1):(s + 1) * (D + 1)],
                              xaug_ap[s])
            alpha = consts.tile([128, NBLOCKS * 128], F32, tag="alpha")
            for k in range(NBLOCKS):
                eng = nc.gpsimd if k % 2 == 0 else nc.sync
                eng.dma_start(alpha[:, k * 128:(k + 1) * 128], alpha_ap[k])
            ones16 = consts.tile([D, 1], BF16, tag="ones")
            nc.vector.memset(ones16[:], 1.0)
            nones16 = consts.tile([D, 1], BF16, tag="nones")
            nc.vector.memset(nones16[:], -1.0)
            ones1 = consts.tile([1, 128], BF16, tag="ones1")
            nc.vector.memset(ones1[:], 1.0)
            ident = consts.tile([128, 128], F32, tag="ident")
            make_identity(nc, ident[:])

            import contextlib
            loop_cm = (tc.For_i(0, iters, 1) if iters > 1
                       else contextlib.nullcontext())
            with loop_cm:
                _kernel_body(nc, tc, mybir, xt16, xt32, xaug, alpha,
                             ones16, nones16, ones1, ident, consts, abs_pool,
                             work, dtaps, dtbps, pops, po2ps, tps, pout_ap)

    nc.compile()
    return nc


def _kernel_body(nc, tc, mybir, xt16, xt32, xaug, alpha, ones16, nones16,
                 ones1, ident, consts, abs_pool, work, dtaps, dtbps, pops,
                 po2ps, tps, pout_ap):
    F32, BF16 = mybir.dt.float32, mybir.dt.bfloat16
    AF = mybir.ActivationFunctionType
    ALU = mybir.AluOpType

    def xaug_blk(s):
        return xaug[:, s * (D + 1):(s + 1) * (D + 1)]

    def finalize(s, acc):
        # O_s = rowsum * X_s - (C@X)_s, straight from the PSUM accumulator
        o = work.tile([128, D], F32, tag="fin")
        nc.vector.scalar_tensor_tensor(
            o[:], xaug_blk(s)[:, 0:D], acc[:, D:D + 1],
            acc[:, 0:D], ALU.mult, ALU.subtract)
        nc.sync.dma_start(pout_ap[s], o[:])

    blk = 0
    po4 = None          # strip-4 PSUM output accumulator (starts at (0,4) mirror)
    for R, jblocks in SCHED:
        nb = len(jblocks)
        jlo = jblocks[0] * 128
        w = nb * 128
        # dt banks: one [128,512] bank for 4 blocks (+ a [128,128] bank for
        # strip 0's 5th block)
        dta = dtaps.tile([128, 512], F32, tag="dta", name=f"dta{R}")
        dtb = (dtbps.tile([128, 128], F32, tag="dtb", name=f"dtb{R}")
               if nb == 5 else None)

        def dt_col(k, i):
            if k < 4:
                return dta[:, k * 128 + i:k * 128 + i + 1]
            return dtb[:, i:i + 1]

        def dt_blk(k):
            if k < 4:
                return dta[:, k * 128:(k + 1) * 128]
            return dtb[:]

        pat = _engine_pattern(*STRIP_SPLIT[R])
        boff = 0 if R == 0 else 128
        for i in range(128):
            gi = boff + i
            if pat[i] == "A":
                ab = abs_pool.tile([D, w], BF16, tag="ab", name=f"ab{R}_{i}")
                nc.scalar.activation(
                    ab[:], xt16[:, jlo:jlo + w], AF.Abs,
                    bias=xt32[:, gi:gi + 1], scale=-1.0)
                for k in range(nb):
                    nc.tensor.matmul(
                        dt_col(k, i),
                        lhsT=ab[:, k * 128:(k + 1) * 128],
                        rhs=ones16[:], start=True, stop=True)
            else:
                pp = abs_pool.tile([D, w], BF16, tag="ab", name=f"pp{R}_{i}")
                nc.vector.tensor_scalar(
                    pp[:], xt16[:, jlo:jlo + w], xt32[:, gi:gi + 1],
                    0.0, ALU.subtract, ALU.max)
                mm = abs_pool.tile([D, w], BF16, tag="ab", name=f"mm{R}_{i}")
                nc.vector.tensor_scalar(
                    mm[:], xt16[:, jlo:jlo + w], xt32[:, gi:gi + 1],
                    0.0, ALU.subtract, ALU.min)
                for k in range(nb):
                    nc.tensor.matmul(
                        dt_col(k, i),
                        lhsT=pp[:, k * 128:(k + 1) * 128],
                        rhs=ones16[:], start=True, stop=False)
                    nc.tensor.matmul(
                        dt_col(k, i),
                        lhsT=mm[:, k * 128:(k + 1) * 128],
                        rhs=nones16[:], start=False, stop=True)
        # dt += 1 in-place via a K=1 rank-1 matmul per block: recip then
        # reads 1+dt straight from PSUM
        USE_PE_PLUS1 = False
        if USE_PE_PLUS1:
            for k in range(nb):
                nc.tensor.matmul(dt_blk(k), lhsT=ones1[:], rhs=ones1[:],
                                 start=False, stop=True, skip_group_check=True)

        # strip output accumulator in PSUM ([C@X | rowsum] summed over blocks)
        if R == 0:
            po = pops.tile([128, D + 1], F32, tag="po", name="po0")
            po_started = False
        else:
            po = po4
            po_started = True

        for k, J in enumerate(jblocks):
            if USE_PE_PLUS1:
                usrc = dt_blk(k)
            else:
                u = work.tile([128, 128], F32, tag="u")
                nc.scalar.add(u[:], dt_blk(k), 1.0)
                usrc = u[:]
            r = work.tile([128, 128], F32, tag="r")
            nc.vector.reciprocal_approx_fast(r[:], usrc)
            ct = work.tile([128, 128], F32, tag="ct")
            nc.gpsimd.tensor_tensor(
                ct[:], r[:], alpha[:, blk * 128:(blk + 1) * 128],
                ALU.mult)
            # O_R partial: accumulate [C@X | rowsum] over this strip's blocks
            nc.tensor.matmul(po[:], lhsT=ct[:], rhs=xaug_blk(J),
                             start=not po_started, stop=(k == nb - 1),
                             skip_group_check=True)
            po_started = True
            if J != R:
                pt = tps.tile([128, 128], F32, tag="pt")
                nc.tensor.transpose(pt[:], ct[:], ident[:])
                ctT = work.tile([128, 128], F32, tag="ctT")
                nc.scalar.copy(ctT[:], pt[:])
                if R == 0 and J == 4:
                    # opens strip 4's accumulation group
                    po4 = pops.tile([128, D + 1], F32, tag="po", name="po4")
                    nc.tensor.matmul(po4[:], lhsT=ctT[:], rhs=xaug_blk(R),
                                     start=True, stop=False,
                                     skip_group_check=True)
                else:
                    po2 = po2ps.tile([128, D + 1], F32, tag="po2")
                    nc.tensor.matmul(po2[:], lhsT=ctT[:], rhs=xaug_blk(R),
                                     start=True, stop=True)
                    finalize(J, po2[:])
            blk += 1
        finalize(R, po[:])


_ALPHA_CACHE = {}


def _core_alpha(q):
    if q in _ALPHA_CACHE:
        return _ALPHA_CACHE[q]
    idx = np.arange(L, dtype=np.float64)
    rot = 128 * q
    real = (idx + rot) % L
    al = np.empty((NBLOCKS, 128, 128), dtype=np.float32)
    k = 0
    for R, jblocks in SCHED:
        ti = real[R * 128:(R + 1) * 128]
        for J in jblocks:
            tj = real[J * 128:(J + 1) * 128]
            dist = np.abs(tj[:, None] - ti[None, :])
            a = 1.0 / (1.0 + dist)
            a[dist == 0] = 0.0
            al[k] = a.astype(np.float32)
            k += 1
    _ALPHA_CACHE[q] = al
    return al


def _prep_host(X):
    """Per-core rotated input dicts. X: [B, L, D] float32."""
    in_maps = []
    for c in range(N_CORES):
        b, q = c // 4, c % 4
        rot = 128 * q
        Xr = np.roll(X[b], -rot, axis=0)          # program token t = real t+rot
        xtT = np.ascontiguousarray(Xr.T)          # [D, L]
        xt16 = xtT.astype(ml_dtypes.bfloat16)
        xt32b = np.ascontiguousarray(
            np.concatenate([xtT[:, 0:128], xtT[:, 512:640]], axis=1))
        xaug = np.concatenate(
            [Xr, np.ones((L, 1), dtype=np.float32)], axis=1)
        xaug = np.ascontiguousarray(xaug.reshape(NBLK, 128, D + 1))
        in_maps.append({"xt16": xt16, "xt32b": xt32b, "xaug": xaug,
                        "alphat": _core_alpha(q)})
    return in_maps


def _get_compiled():
    global _COMPILED
    if _COMPILED is None:
        _COMPILED = _build()
    return _COMPILED


def kernel(X, _trace=False, _trace_kwargs=None):
    """X: np.ndarray [2, 1024, 128] float32 -> O [2, 1024, 128] float32."""
    from concourse.bass_utils import run_bass_kernel_spmd

    X = np.asarray(X, dtype=np.float32)
    assert X.shape == (B, L, D)
    nc = _get_compiled()
    in_maps = _prep_host(X)
    res = run_bass_kernel_spmd(nc, in_maps, list(range(N_CORES)),
                               trace=_trace, **(_trace_kwargs or {}))
    O = np.zeros((B, L, D), dtype=np.float32)
    for c in range(N_CORES):
        b, q = c // 4, c % 4
        part = res.results[c]["pout"].reshape(L, D)
        O[b] += np.roll(part, 128 * q, axis=0)    # un-rotate
    if _trace:
        return O, res
    return O


if __name__ == "__main__":
    rng = np.random.default_rng(0)
    X = rng.standard_normal((B, L, D), dtype=np.float32)
    O = kernel(X)
    print("ok", O.shape, float(np.abs(O).max()))
